# revision 2
# baseline (speedup 1.0000x reference)
"""Trainium2 Bass kernel for the LDPC min-sum decoder problem
(nn_Decoding_model: 10 iterations of min-sum message passing + CE loss).

Sharding: pure data parallel — batch item b runs on NeuronCore b (8 cores).
H and the normalizer are replicated; the scalar loss partials are summed on
the host during output assembly.

Device algorithm (per core, all standard PE/DVE/ACT instructions):
- Row r -> partition p = r % 128, slot t = r // 128; edge (r, j) at
  f = 6t + j (48 edge slots per partition; 6144 edges exactly).
- total kept as T16 [128, 16]: T16[ph, cl] = total[16*ph + cl].
- Gather (total -> edges): per slot f a one-hot matmul
  cand_f[p, cl] = T16[sigma_f(p), cl], then a mask-multiply + strided sum
  selects cl(p, f).
- Check-node update (min/min2 tournament, sign product) entirely on
  compact [128, 48] tiles.
- Scatter (edges -> columns): per slot f a one-hot matmul accumulated in
  PSUM: colsum16[cl, ph] += sum_p cv16[p, f, cl] * [sigma_f(p) = ph] —
  a true scatter-add, duplicate columns just accumulate.
- Sigmoid-CE loss computed on-device from the 10 stored outputs.
"""
import numpy as np

ROWS, COLS, ROW_W = 1024, 2048, 6
BATCH = 8
NUM_ITERS = 10
NSLOT = 48
CL = 16

_CACHE = {}


# ---------------------------------------------------------------------------
# Tile drain patch: this walrus build accepts only one sync-wait command per
# CTRL (drain/nop) instruction, but Tile attaches the whole global clock to
# the final drain.  Split the waits across several nops.
# ---------------------------------------------------------------------------
def _apply_tile_patch():
    import concourse.tile as ctile
    from concourse.vector_clock import ScopedClock
    from concourse import mybir as _mb

    if getattr(ctile.TileContext, "_drain_patched", False):
        return

    def _patched(self, tick_clock, wait_clock):
        nc = self.nc
        carrier = nc.sync.nop(nofuse=True)
        wait_clock.add_sem_waits(
            carrier.ins, ScopedClock({None: tick_clock.global_clock}))
        si = carrier.ins.sync_info
        if si is not None and si.on_wait is not None and len(si.on_wait) > 1:
            waits = list(si.on_wait)
            si.on_wait = waits[:1]
            for i in range(1, len(waits)):
                extra = nc.sync.nop(nofuse=True)
                extra.ins.sync_info = _mb.SyncInfo(
                    on_wait=[waits[i]], on_update=[])
        nc.sync.drain()
        nc.all_engine_barrier()
        popped = nc._tile_sem_poison_stack.pop()
        assert popped is self._sem_poison
        nc.clear_and_free_semaphores(list(self.sems.allocated().values()))
        nc.all_engine_barrier()

    ctile.TileContext._drain_and_barrier = _patched
    ctile.TileContext._drain_patched = True


# ---------------------------------------------------------------------------
# Host preprocessing: one-hot routing tables from H
# ---------------------------------------------------------------------------
def _build_tables(H):
    idx = np.argsort(-H, axis=1, kind="stable")[:, :ROW_W]
    idx = np.sort(idx, axis=1).astype(np.int64)
    OHT = np.zeros((128, NSLOT * 128), dtype=np.float32)
    OHB = np.zeros((128, NSLOT * 128), dtype=np.float32)
    CLm = np.zeros((128, NSLOT * CL), dtype=np.float32)
    for p in range(128):
        for t in range(8):
            r = t * 128 + p
            for j in range(ROW_W):
                f = 6 * t + j
                c = int(idx[r, j])
                ph, cl = c // 16, c % 16
                OHT[ph, 128 * f + p] = 1.0
                OHB[p, 128 * f + ph] = 1.0
                CLm[p, CL * f + cl] = 1.0
    return OHT, OHB, CLm


# ---------------------------------------------------------------------------
# Bass kernel builder
# ---------------------------------------------------------------------------
def _build_nc(num_iters=NUM_ITERS):
    import concourse.bass as bass
    import concourse.mybir as mybir
    from concourse.tile import TileContext

    F32 = mybir.dt.float32
    ALU = mybir.AluOpType
    ACTF = mybir.ActivationFunctionType

    nc = bass.Bass("TRN2")
    soft16 = nc.dram_tensor("soft16", [128, 16], F32, kind="ExternalInput")
    lab16 = nc.dram_tensor("lab16", [128, 16], F32, kind="ExternalInput")
    gnorm = nc.dram_tensor("gnorm", [1, 1], F32, kind="ExternalInput")
    OHT = nc.dram_tensor("OHT", [128, NSLOT * 128], F32, kind="ExternalInput")
    OHB = nc.dram_tensor("OHB", [128, NSLOT * 128], F32, kind="ExternalInput")
    CLmI = nc.dram_tensor("CLm", [128, NSLOT * CL], F32, kind="ExternalInput")
    identI = nc.dram_tensor("ident16", [16, 16], F32, kind="ExternalInput")
    outs = nc.dram_tensor("outs", [num_iters, 128, 16], F32,
                          kind="ExternalOutput")
    lossp = nc.dram_tensor("lossp", [1, 1], F32, kind="ExternalOutput")

    with TileContext(nc) as tc:
        with tc.tile_pool(name="const", bufs=1) as cp, \
             tc.tile_pool(name="state", bufs=1) as sp, \
             tc.tile_pool(name="work", bufs=2) as wp, \
             tc.tile_pool(name="dram", bufs=1, space="DRAM") as dp, \
             tc.tile_pool(name="psA", bufs=2, space="PSUM") as psA, \
             tc.tile_pool(name="psB", bufs=2, space="PSUM") as psB:

            oht_t = cp.tile([128, NSLOT * 128], F32)
            ohb_t = cp.tile([128, NSLOT * 128], F32)
            clm_t = cp.tile([128, NSLOT * CL], F32)
            sft_t = cp.tile([128, 16], F32)
            lab_t = cp.tile([128, 16], F32)
            gn_t = cp.tile([1, 1], F32)
            ones1_128 = cp.tile([1, 128], F32)
            ident16 = cp.tile([16, 16], F32)
            ones128 = cp.tile([128, 1], F32)
            gamma_t = cp.tile([128, 1], F32)

            nc.sync.dma_start(oht_t[:], OHT[:])
            nc.sync.dma_start(ohb_t[:], OHB[:])
            nc.sync.dma_start(clm_t[:], CLmI[:])
            nc.sync.dma_start(sft_t[:], soft16[:])
            nc.sync.dma_start(lab_t[:], lab16[:])
            nc.sync.dma_start(gn_t[:], gnorm[:])
            nc.sync.dma_start(ident16[:], identI[:])
            nc.vector.memset(ones1_128[:], 1.0)
            nc.vector.memset(ones128[:], 1.0)

            gam1 = cp.tile([1, 1], F32)
            gex = cp.tile([1, 1], F32)
            nc.scalar.activation(gex[:], gn_t[:], ACTF.Exp)
            nc.vector.tensor_scalar_add(gex[:], gex[:], 1.0)
            nc.scalar.activation(gam1[:], gex[:], ACTF.Ln)
            gam_ps = psB.tile([128, 1], F32, tag="cs")
            nc.tensor.matmul(gam_ps[:], ones1_128[:], gam1[:],
                             start=True, stop=True)
            nc.scalar.copy(gamma_t[:], gam_ps[:])

            T16 = sp.tile([128, 16], F32)
            cvR = sp.tile([128, NSLOT], F32)
            outs_mir = dp.tile([num_iters, 128, 16], F32)
            nc.vector.memset(cvR[:], 0.0)
            nc.vector.tensor_copy(T16[:], sft_t[:])

            for it in range(num_iters):
                # gather candidates: cand[p, 16f+cl] = T16[sigma_f(p), cl]
                cand = psA.tile([128, NSLOT * CL], F32, tag="cand")
                for f in range(NSLOT):
                    nc.tensor.matmul(cand[:, 16 * f : 16 * f + 16],
                                     oht_t[:, 128 * f : 128 * f + 128],
                                     T16[:], start=True, stop=True)
                mm = wp.tile([128, NSLOT * CL], F32, tag="mm")
                nc.vector.tensor_tensor(mm[:], cand[:], clm_t[:], ALU.mult)
                tg = wp.tile([128, NSLOT], F32, tag="tg")
                nc.vector.tensor_reduce(
                    tg[:], mm[:].rearrange("p (f c) -> p f c", c=CL),
                    axis=mybir.AxisListType.X, op=ALU.add)

                # edge math (row-compact [128, 48])
                vc = wp.tile([128, NSLOT], F32, tag="vc")
                nc.vector.tensor_tensor(vc[:], tg[:], cvR[:], ALU.subtract)
                av = wp.tile([128, NSLOT], F32, tag="av")
                nc.scalar.activation(av[:], vc[:], ACTF.Abs)
                s = wp.tile([128, NSLOT], F32, tag="s")
                nc.scalar.sign(s[:], vc[:])

                av3 = av[:].rearrange("p (t j) -> p t j", j=6)
                s3 = s[:].rearrange("p (t j) -> p t j", j=6)

                mnp = wp.tile([128, 8, 3], F32, tag="mnp")
                mxp = wp.tile([128, 8, 3], F32, tag="mxp")
                nc.vector.tensor_tensor(mnp[:], av3[:, :, 0::2],
                                        av3[:, :, 1::2], ALU.min)
                nc.vector.tensor_tensor(mxp[:], av3[:, :, 0::2],
                                        av3[:, :, 1::2], ALU.max)
                mn01 = wp.tile([128, 8], F32, tag="mn01")
                mx01 = wp.tile([128, 8], F32, tag="mx01")
                t01 = wp.tile([128, 8], F32, tag="t01")
                mn2_01 = wp.tile([128, 8], F32, tag="mn2_01")
                nc.vector.tensor_tensor(mn01[:], mnp[:, :, 0], mnp[:, :, 1],
                                        ALU.min)
                nc.vector.tensor_tensor(mx01[:], mnp[:, :, 0], mnp[:, :, 1],
                                        ALU.max)
                nc.vector.tensor_tensor(t01[:], mxp[:, :, 0], mxp[:, :, 1],
                                        ALU.min)
                nc.vector.tensor_tensor(mn2_01[:], mx01[:], t01[:], ALU.min)
                mn = wp.tile([128, 8], F32, tag="mn")
                mxL = wp.tile([128, 8], F32, tag="mxL")
                t2 = wp.tile([128, 8], F32, tag="t2")
                mn2 = wp.tile([128, 8], F32, tag="mn2")
                nc.vector.tensor_tensor(mn[:], mn01[:], mnp[:, :, 2], ALU.min)
                nc.vector.tensor_tensor(mxL[:], mn01[:], mnp[:, :, 2], ALU.max)
                nc.vector.tensor_tensor(t2[:], mn2_01[:], mxp[:, :, 2], ALU.min)
                nc.vector.tensor_tensor(mn2[:], mxL[:], t2[:], ALU.min)

                sp2 = wp.tile([128, 8, 3], F32, tag="sp2")
                nc.vector.tensor_tensor(sp2[:], s3[:, :, 0::2], s3[:, :, 1::2],
                                        ALU.mult)
                sp1 = wp.tile([128, 8], F32, tag="sp1")
                nc.vector.tensor_tensor(sp1[:], sp2[:, :, 0], sp2[:, :, 1],
                                        ALU.mult)
                rs = wp.tile([128, 8], F32, tag="rs")
                nc.vector.tensor_tensor(rs[:], sp1[:], sp2[:, :, 2], ALU.mult)
                grs = wp.tile([128, 8], F32, tag="grs")
                nc.vector.tensor_scalar_mul(grs[:], rs[:], gamma_t[:])
                M1 = wp.tile([128, 8], F32, tag="M1")
                M2 = wp.tile([128, 8], F32, tag="M2")
                nc.vector.tensor_tensor(M1[:], grs[:], mn[:], ALU.mult)
                nc.vector.tensor_tensor(M2[:], grs[:], mn2[:], ALU.mult)

                mn_b = mn[:].broadcast_to([128, 8, 6])
                M1_b = M1[:].broadcast_to([128, 8, 6])
                ge = wp.tile([128, NSLOT], F32, tag="ge")
                nc.vector.tensor_tensor(
                    ge[:].rearrange("p (t j) -> p t j", j=6), av3, mn_b,
                    ALU.is_le)
                D = wp.tile([128, 8], F32, tag="D")
                nc.vector.tensor_tensor(D[:], M2[:], M1[:], ALU.subtract)
                D_b = D[:].broadcast_to([128, 8, 6])
                gd = wp.tile([128, NSLOT], F32, tag="gd")
                nc.vector.tensor_tensor(
                    gd[:].rearrange("p (t j) -> p t j", j=6),
                    ge[:].rearrange("p (t j) -> p t j", j=6), D_b, ALU.mult)
                r = wp.tile([128, NSLOT], F32, tag="r")
                nc.vector.tensor_tensor(
                    r[:].rearrange("p (t j) -> p t j", j=6),
                    gd[:].rearrange("p (t j) -> p t j", j=6), M1_b, ALU.add)
                cvn = wp.tile([128, NSLOT], F32, tag="cvn")
                nc.vector.tensor_tensor(cvn[:], r[:], s[:], ALU.mult)
                nc.vector.tensor_copy(cvR[:], cvn[:])

                # scatter: colsum16[cl, ph] += cv16_f^T @ OHB_f (PSUM accum)
                cv16 = wp.tile([128, NSLOT * CL], F32, tag="cv16")
                nc.vector.tensor_tensor(
                    cv16[:].rearrange("p (f c) -> p f c", c=CL),
                    cvn[:].broadcast_to([128, NSLOT, CL]),
                    clm_t[:].rearrange("p (f c) -> p f c", c=CL), ALU.mult)
                cs16 = psB.tile([16, 128], F32, tag="cs")
                for f in range(NSLOT):
                    nc.tensor.matmul(cs16[:], cv16[:, 16 * f : 16 * f + 16],
                                     ohb_t[:, 128 * f : 128 * f + 128],
                                     start=(f == 0), stop=(f == NSLOT - 1))
                csb = wp.tile([16, 128], F32, tag="csb")
                nc.vector.tensor_copy(csb[:], cs16[:])
                trp = psA.tile([128, 16], F32, tag="trp")
                nc.tensor.matmul(trp[:], csb[:], ident16[:],
                                 start=True, stop=True)
                nc.vector.tensor_tensor(T16[:], trp[:], sft_t[:], ALU.add)
                nc.sync.dma_start(outs[it], T16[:])
                nc.scalar.dma_start(outs_mir[it], T16[:])

            # CE loss over all iterations
            z = wp.tile([128, 16 * num_iters], F32, tag="z")
            zsrc = outs_mir[:].rearrange("i p c -> p c i")
            nc.sync.dma_start(z[:].rearrange("p (c i) -> p c i", c=16), zsrc)
            lab_b = lab_t[:].broadcast_to([128, 16, num_iters])
            z3 = z[:].rearrange("p (c i) -> p c i", c=16)
            t1 = wp.tile([128, 16 * num_iters], F32, tag="t1")
            nc.scalar.activation(t1[:], z[:], ACTF.Relu, scale=-1.0)
            az = wp.tile([128, 16 * num_iters], F32, tag="az")
            nc.scalar.activation(az[:], z[:], ACTF.Abs)
            e = wp.tile([128, 16 * num_iters], F32, tag="e")
            nc.scalar.activation(e[:], az[:], ACTF.Exp, scale=-1.0)
            ep1 = wp.tile([128, 16 * num_iters], F32, tag="ep1")
            nc.vector.tensor_scalar_add(ep1[:], e[:], 1.0)
            lg = wp.tile([128, 16 * num_iters], F32, tag="lg")
            nc.scalar.activation(lg[:], ep1[:], ACTF.Ln)
            zl = wp.tile([128, 16 * num_iters], F32, tag="zl")
            nc.vector.tensor_tensor(zl[:].rearrange("p (c i) -> p c i", c=16),
                                    z3, lab_b, ALU.mult)
            u = wp.tile([128, 16 * num_iters], F32, tag="u")
            nc.vector.tensor_tensor(u[:], t1[:], zl[:], ALU.add)
            v = wp.tile([128, 16 * num_iters], F32, tag="v")
            nc.vector.tensor_tensor(v[:], u[:], lg[:], ALU.add)
            lv = wp.tile([128, 1], F32, tag="lv")
            nc.vector.tensor_reduce(lv[:], v[:], axis=mybir.AxisListType.X,
                                    op=ALU.add)
            loss_ps = psB.tile([1, 1], F32, tag="cs")
            nc.tensor.matmul(loss_ps[:], lv[:], ones128[:],
                             start=True, stop=True)
            losssb = wp.tile([1, 1], F32, tag="losssb")
            nc.scalar.copy(losssb[:], loss_ps[:])
            nc.sync.dma_start(lossp[:], losssb[:])

    return nc


def _split_sync_waits(nc, maxw=1):
    """This walrus build accepts at most one sync-wait command per
    instruction; hoist extra waits onto same-engine nops placed before."""
    import concourse.mybir as mybir

    cnt = 0
    for f in nc.m.functions:
        for bb in f.blocks:
            newlist = []
            for inst in bb.instructions:
                si = inst.sync_info
                if si is not None and si.on_wait is not None \
                        and len(si.on_wait) > maxw:
                    waits = list(si.on_wait)
                    si.on_wait = waits[-maxw:]
                    for w in waits[:-maxw]:
                        cnt += 1
                        nop = mybir.InstNoOp(name=f"I-waitsplit-{cnt}",
                                             ins=[], outs=[])
                        nop.engine = inst.engine
                        nop.sync_info = mybir.SyncInfo(on_wait=[w],
                                                       on_update=[])
                        newlist.append(nop)
                newlist.append(inst)
            if cnt:
                bb.instructions[:] = newlist
    return nc


def _get_compiled():
    if "nc" not in _CACHE:
        _apply_tile_patch()
        _CACHE["nc"] = _split_sync_waits(_build_nc())
    return _CACHE["nc"]


def kernel(soft_input, labels, H, normalizer, _trace=False):
    from concourse import bass_utils

    soft_input = np.asarray(soft_input, dtype=np.float32)
    labels_np = np.asarray(labels)
    H = np.asarray(H)
    normalizer = np.asarray(normalizer, dtype=np.float32)

    hkey = ("tables", H.shape, int(H.sum()), int(H[0].argmax()))
    if hkey not in _CACHE:
        _CACHE[hkey] = _build_tables(H)
    OHT, OHB, CLm = _CACHE[hkey]
    nc = _get_compiled()

    ident = np.eye(16, dtype=np.float32)
    gn = normalizer.reshape(1, 1)
    in_maps = []
    for b in range(BATCH):
        in_maps.append({
            "soft16": soft_input[b].reshape(128, 16),
            "lab16": labels_np[b].astype(np.float32).reshape(128, 16),
            "gnorm": gn,
            "OHT": OHT,
            "OHB": OHB,
            "CLm": CLm,
            "ident16": ident,
        })

    res = bass_utils.run_bass_kernel_spmd(
        nc, in_maps, core_ids=list(range(BATCH)), trace=_trace)

    stacked = np.zeros((NUM_ITERS + 1, BATCH, COLS), dtype=np.float32)
    stacked[0] = soft_input
    loss = np.float32(0.0)
    for b in range(BATCH):
        stacked[1:, b, :] = res.results[b]["outs"].reshape(NUM_ITERS, COLS)
        loss += np.float32(res.results[b]["lossp"][0, 0])
    out = (stacked, labels_np, np.float32(loss))
    if _trace:
        return out, res
    return out


# revision 4
# speedup vs baseline: 1.5085x; 1.5085x over previous
"""Trainium2 Bass kernel for the LDPC min-sum decoder problem
(nn_Decoding_model: 10 iterations of min-sum message passing + CE loss).

Sharding: pure data parallel — batch item b runs on NeuronCore b (8 cores).
H and the normalizer are replicated; the scalar loss partials are summed on
the host during output assembly.

Device algorithm (per core, all standard PE/DVE/ACT instructions):
- Row r -> partition p = r % 128, slot t = r // 128; edge (r, j) at
  f = 6t + j (48 edge slots per partition; 6144 edges exactly).
- total kept as T16 [128, 16]: T16[ph, cl] = total[16*ph + cl].
- Gather (total -> edges): per slot f a one-hot matmul
  cand_f[p, cl] = T16[sigma_f(p), cl], then a mask-multiply + strided sum
  selects cl(p, f).
- Check-node update (min/min2 tournament, sign product) entirely on
  compact [128, 48] tiles.
- Scatter (edges -> columns): per slot f a one-hot matmul accumulated in
  PSUM: colsum16[cl, ph] += sum_p cv16[p, f, cl] * [sigma_f(p) = ph] —
  a true scatter-add, duplicate columns just accumulate.
- Sigmoid-CE loss computed on-device from the 10 stored outputs.
"""
import numpy as np

ROWS, COLS, ROW_W = 1024, 2048, 6
BATCH = 8
NUM_ITERS = 10
NSLOT = 48
CL = 16

_CACHE = {}


# ---------------------------------------------------------------------------
# Tile drain patch: this walrus build accepts only one sync-wait command per
# CTRL (drain/nop) instruction, but Tile attaches the whole global clock to
# the final drain.  Split the waits across several nops.
# ---------------------------------------------------------------------------
def _apply_tile_patch():
    import concourse.tile as ctile
    from concourse.vector_clock import ScopedClock
    from concourse import mybir as _mb

    if getattr(ctile.TileContext, "_drain_patched", False):
        return

    def _patched(self, tick_clock, wait_clock):
        nc = self.nc
        carrier = nc.sync.nop(nofuse=True)
        wait_clock.add_sem_waits(
            carrier.ins, ScopedClock({None: tick_clock.global_clock}))
        si = carrier.ins.sync_info
        if si is not None and si.on_wait is not None and len(si.on_wait) > 1:
            waits = list(si.on_wait)
            si.on_wait = waits[:1]
            for i in range(1, len(waits)):
                extra = nc.sync.nop(nofuse=True)
                extra.ins.sync_info = _mb.SyncInfo(
                    on_wait=[waits[i]], on_update=[])
        nc.sync.drain()
        nc.all_engine_barrier()
        popped = nc._tile_sem_poison_stack.pop()
        assert popped is self._sem_poison
        nc.clear_and_free_semaphores(list(self.sems.allocated().values()))
        nc.all_engine_barrier()

    ctile.TileContext._drain_and_barrier = _patched
    ctile.TileContext._drain_patched = True


# ---------------------------------------------------------------------------
# Host preprocessing: one-hot routing tables from H
# ---------------------------------------------------------------------------
def _build_tables(H):
    idx = np.argsort(-H, axis=1, kind="stable")[:, :ROW_W]
    idx = np.sort(idx, axis=1).astype(np.int64)
    OHT = np.zeros((128, NSLOT * 128), dtype=np.float32)
    OHB = np.zeros((128, NSLOT * 128), dtype=np.float32)
    CLm = np.zeros((128, NSLOT * CL), dtype=np.float32)
    for p in range(128):
        for t in range(8):
            r = t * 128 + p
            for j in range(ROW_W):
                f = 6 * t + j
                c = int(idx[r, j])
                ph, cl = c // 16, c % 16
                OHT[ph, 128 * f + p] = 1.0
                OHB[p, 128 * f + ph] = 1.0
                CLm[p, CL * f + cl] = 1.0
    return OHT, OHB, CLm


# ---------------------------------------------------------------------------
# Bass kernel builder
# ---------------------------------------------------------------------------
def _build_nc(num_iters=NUM_ITERS):
    import concourse.bass as bass
    import concourse.mybir as mybir
    from concourse.tile import TileContext

    F32 = mybir.dt.float32
    BF16 = mybir.dt.bfloat16
    ALU = mybir.AluOpType
    ACTF = mybir.ActivationFunctionType

    nc = bass.Bass("TRN2")
    soft16 = nc.dram_tensor("soft16", [128, 16], F32, kind="ExternalInput")
    lab16 = nc.dram_tensor("lab16", [128, 16], F32, kind="ExternalInput")
    gnorm = nc.dram_tensor("gnorm", [1, 1], F32, kind="ExternalInput")
    OHT = nc.dram_tensor("OHT", [128, NSLOT * 128], BF16, kind="ExternalInput")
    OHB = nc.dram_tensor("OHB", [128, NSLOT * 128], F32, kind="ExternalInput")
    CLmI = nc.dram_tensor("CLm", [128, NSLOT * CL], F32, kind="ExternalInput")
    CLm2I = nc.dram_tensor("CLm2", [128, NSLOT * 2 * CL], F32,
                           kind="ExternalInput")
    identI = nc.dram_tensor("ident16", [16, 16], F32, kind="ExternalInput")
    outs = nc.dram_tensor("outs", [num_iters, 128, 16], F32,
                          kind="ExternalOutput")
    lossp = nc.dram_tensor("lossp", [1, 1], F32, kind="ExternalOutput")

    with TileContext(nc) as tc:
        with tc.tile_pool(name="const", bufs=1) as cp, \
             tc.tile_pool(name="state", bufs=1) as sp, \
             tc.tile_pool(name="work", bufs=2) as wp, \
             tc.tile_pool(name="dram", bufs=1, space="DRAM") as dp, \
             tc.tile_pool(name="psA", bufs=1, space="PSUM") as psA, \
             tc.tile_pool(name="psB", bufs=2, space="PSUM") as psB:

            oht_t = cp.tile([128, NSLOT * 128], BF16)
            ohb_t = cp.tile([128, NSLOT * 128], F32)
            clm_t = cp.tile([128, NSLOT * CL], F32)
            clm2_t = cp.tile([128, NSLOT * 2 * CL], F32)
            sft_t = cp.tile([128, 16], F32)
            lab_t = cp.tile([128, 16], F32)
            gn_t = cp.tile([1, 1], F32)
            ones1_128 = cp.tile([1, 128], F32)
            ident16 = cp.tile([16, 16], F32)
            ones128 = cp.tile([128, 1], F32)
            gamma_t = cp.tile([128, 1], F32)

            nc.sync.dma_start(oht_t[:], OHT[:])
            nc.sync.dma_start(ohb_t[:], OHB[:])
            nc.sync.dma_start(clm_t[:], CLmI[:])
            nc.sync.dma_start(clm2_t[:], CLm2I[:])
            nc.sync.dma_start(sft_t[:], soft16[:])
            nc.sync.dma_start(lab_t[:], lab16[:])
            nc.sync.dma_start(gn_t[:], gnorm[:])
            nc.sync.dma_start(ident16[:], identI[:])
            nc.vector.memset(ones1_128[:], 1.0)
            nc.vector.memset(ones128[:], 1.0)

            gam1 = cp.tile([1, 1], F32)
            gex = cp.tile([1, 1], F32)
            nc.scalar.activation(gex[:], gn_t[:], ACTF.Exp)
            nc.vector.tensor_scalar_add(gex[:], gex[:], 1.0)
            nc.scalar.activation(gam1[:], gex[:], ACTF.Ln)
            gam_ps = psB.tile([128, 1], F32, tag="cs")
            nc.tensor.matmul(gam_ps[:], ones1_128[:], gam1[:],
                             start=True, stop=True)
            nc.scalar.copy(gamma_t[:], gam_ps[:])

            T16 = sp.tile([128, 16], F32)
            cvR = sp.tile([128, NSLOT], F32)
            outs_mir = dp.tile([num_iters, 128, 16], F32)
            nc.vector.memset(cvR[:], 0.0)
            nc.vector.tensor_copy(T16[:], sft_t[:])

            for it in range(num_iters):
                # bf16 hi/lo split of T16 (exact to ~2^-18 relative)
                thl = wp.tile([128, 2 * CL], BF16, tag="thl")
                nc.vector.tensor_copy(thl[:, 0:CL], T16[:])
                hi32 = wp.tile([128, CL], F32, tag="hi32")
                nc.vector.tensor_copy(hi32[:], thl[:, 0:CL])
                nc.vector.tensor_tensor(thl[:, CL : 2 * CL], T16[:], hi32[:],
                                        ALU.subtract)
                # gather candidates (hi|lo packed in N):
                # cand[p, 32f + {0..15|16..31}] = T16{hi|lo}[sigma_f(p), cl]
                cand = psA.tile([128, NSLOT * 2 * CL], F32, tag="cand")
                for f in range(NSLOT):
                    nc.tensor.matmul(cand[:, 32 * f : 32 * f + 32],
                                     oht_t[:, 128 * f : 128 * f + 128],
                                     thl[:], start=True, stop=True)
                mm = wp.tile([128, NSLOT * 2 * CL], F32, tag="mm")
                nc.vector.tensor_tensor(mm[:], cand[:], clm2_t[:], ALU.mult)
                tg = wp.tile([128, NSLOT], F32, tag="tg")
                nc.vector.tensor_reduce(
                    tg[:], mm[:].rearrange("p (f c) -> p f c", c=2 * CL),
                    axis=mybir.AxisListType.X, op=ALU.add)

                # edge math (row-compact [128, 48])
                vc = wp.tile([128, NSLOT], F32, tag="vc")
                nc.vector.tensor_tensor(vc[:], tg[:], cvR[:], ALU.subtract)
                av = wp.tile([128, NSLOT], F32, tag="av")
                nc.scalar.activation(av[:], vc[:], ACTF.Abs)
                s = wp.tile([128, NSLOT], F32, tag="s")
                nc.scalar.sign(s[:], vc[:])

                av3 = av[:].rearrange("p (t j) -> p t j", j=6)
                s3 = s[:].rearrange("p (t j) -> p t j", j=6)

                mnp = wp.tile([128, 8, 3], F32, tag="mnp")
                mxp = wp.tile([128, 8, 3], F32, tag="mxp")
                nc.vector.tensor_tensor(mnp[:], av3[:, :, 0::2],
                                        av3[:, :, 1::2], ALU.min)
                nc.vector.tensor_tensor(mxp[:], av3[:, :, 0::2],
                                        av3[:, :, 1::2], ALU.max)
                mn01 = wp.tile([128, 8], F32, tag="mn01")
                mx01 = wp.tile([128, 8], F32, tag="mx01")
                t01 = wp.tile([128, 8], F32, tag="t01")
                mn2_01 = wp.tile([128, 8], F32, tag="mn2_01")
                nc.vector.tensor_tensor(mn01[:], mnp[:, :, 0], mnp[:, :, 1],
                                        ALU.min)
                nc.vector.tensor_tensor(mx01[:], mnp[:, :, 0], mnp[:, :, 1],
                                        ALU.max)
                nc.vector.tensor_tensor(t01[:], mxp[:, :, 0], mxp[:, :, 1],
                                        ALU.min)
                nc.vector.tensor_tensor(mn2_01[:], mx01[:], t01[:], ALU.min)
                mn = wp.tile([128, 8], F32, tag="mn")
                mxL = wp.tile([128, 8], F32, tag="mxL")
                t2 = wp.tile([128, 8], F32, tag="t2")
                mn2 = wp.tile([128, 8], F32, tag="mn2")
                nc.vector.tensor_tensor(mn[:], mn01[:], mnp[:, :, 2], ALU.min)
                nc.vector.tensor_tensor(mxL[:], mn01[:], mnp[:, :, 2], ALU.max)
                nc.vector.tensor_tensor(t2[:], mn2_01[:], mxp[:, :, 2], ALU.min)
                nc.vector.tensor_tensor(mn2[:], mxL[:], t2[:], ALU.min)

                sp2 = wp.tile([128, 8, 3], F32, tag="sp2")
                nc.vector.tensor_tensor(sp2[:], s3[:, :, 0::2], s3[:, :, 1::2],
                                        ALU.mult)
                sp1 = wp.tile([128, 8], F32, tag="sp1")
                nc.vector.tensor_tensor(sp1[:], sp2[:, :, 0], sp2[:, :, 1],
                                        ALU.mult)
                rs = wp.tile([128, 8], F32, tag="rs")
                nc.vector.tensor_tensor(rs[:], sp1[:], sp2[:, :, 2], ALU.mult)
                grs = wp.tile([128, 8], F32, tag="grs")
                nc.vector.tensor_scalar_mul(grs[:], rs[:], gamma_t[:])
                M1 = wp.tile([128, 8], F32, tag="M1")
                M2 = wp.tile([128, 8], F32, tag="M2")
                nc.vector.tensor_tensor(M1[:], grs[:], mn[:], ALU.mult)
                nc.vector.tensor_tensor(M2[:], grs[:], mn2[:], ALU.mult)

                mn_b = mn[:].broadcast_to([128, 8, 6])
                M1_b = M1[:].broadcast_to([128, 8, 6])
                ge = wp.tile([128, NSLOT], F32, tag="ge")
                nc.vector.tensor_tensor(
                    ge[:].rearrange("p (t j) -> p t j", j=6), av3, mn_b,
                    ALU.is_le)
                D = wp.tile([128, 8], F32, tag="D")
                nc.vector.tensor_tensor(D[:], M2[:], M1[:], ALU.subtract)
                D_b = D[:].broadcast_to([128, 8, 6])
                gd = wp.tile([128, NSLOT], F32, tag="gd")
                nc.vector.tensor_tensor(
                    gd[:].rearrange("p (t j) -> p t j", j=6),
                    ge[:].rearrange("p (t j) -> p t j", j=6), D_b, ALU.mult)
                r = wp.tile([128, NSLOT], F32, tag="r")
                nc.vector.tensor_tensor(
                    r[:].rearrange("p (t j) -> p t j", j=6),
                    gd[:].rearrange("p (t j) -> p t j", j=6), M1_b, ALU.add)
                cvn = wp.tile([128, NSLOT], F32, tag="cvn")
                nc.vector.tensor_tensor(cvn[:], r[:], s[:], ALU.mult)
                nc.vector.tensor_copy(cvR[:], cvn[:])

                # scatter: colsum16[cl, ph] += cv16_f^T @ OHB_f (PSUM accum)
                cv16 = wp.tile([128, NSLOT * CL], F32, tag="cv16")
                nc.vector.tensor_tensor(
                    cv16[:].rearrange("p (f c) -> p f c", c=CL),
                    cvn[:].broadcast_to([128, NSLOT, CL]),
                    clm_t[:].rearrange("p (f c) -> p f c", c=CL), ALU.mult)
                cs16 = psB.tile([16, 128], F32, tag="cs")
                for f in range(NSLOT):
                    nc.tensor.matmul(cs16[:], cv16[:, 16 * f : 16 * f + 16],
                                     ohb_t[:, 128 * f : 128 * f + 128],
                                     start=(f == 0), stop=(f == NSLOT - 1))
                csb = wp.tile([16, 128], F32, tag="csb")
                nc.vector.tensor_copy(csb[:], cs16[:])
                trp = psA.tile([128, 16], F32, tag="trp")
                nc.tensor.matmul(trp[:], csb[:], ident16[:],
                                 start=True, stop=True)
                nc.vector.tensor_tensor(T16[:], trp[:], sft_t[:], ALU.add)
                nc.sync.dma_start(outs[it], T16[:])
                nc.scalar.dma_start(outs_mir[it], T16[:])

            # CE loss over all iterations
            z = wp.tile([128, 16 * num_iters], F32, tag="z")
            zsrc = outs_mir[:].rearrange("i p c -> p c i")
            nc.sync.dma_start(z[:].rearrange("p (c i) -> p c i", c=16), zsrc)
            lab_b = lab_t[:].broadcast_to([128, 16, num_iters])
            z3 = z[:].rearrange("p (c i) -> p c i", c=16)
            t1 = wp.tile([128, 16 * num_iters], F32, tag="t1")
            nc.scalar.activation(t1[:], z[:], ACTF.Relu, scale=-1.0)
            az = wp.tile([128, 16 * num_iters], F32, tag="az")
            nc.scalar.activation(az[:], z[:], ACTF.Abs)
            e = wp.tile([128, 16 * num_iters], F32, tag="e")
            nc.scalar.activation(e[:], az[:], ACTF.Exp, scale=-1.0)
            ep1 = wp.tile([128, 16 * num_iters], F32, tag="ep1")
            nc.vector.tensor_scalar_add(ep1[:], e[:], 1.0)
            lg = wp.tile([128, 16 * num_iters], F32, tag="lg")
            nc.scalar.activation(lg[:], ep1[:], ACTF.Ln)
            zl = wp.tile([128, 16 * num_iters], F32, tag="zl")
            nc.vector.tensor_tensor(zl[:].rearrange("p (c i) -> p c i", c=16),
                                    z3, lab_b, ALU.mult)
            u = wp.tile([128, 16 * num_iters], F32, tag="u")
            nc.vector.tensor_tensor(u[:], t1[:], zl[:], ALU.add)
            v = wp.tile([128, 16 * num_iters], F32, tag="v")
            nc.vector.tensor_tensor(v[:], u[:], lg[:], ALU.add)
            lv = wp.tile([128, 1], F32, tag="lv")
            nc.vector.tensor_reduce(lv[:], v[:], axis=mybir.AxisListType.X,
                                    op=ALU.add)
            loss_ps = psB.tile([1, 1], F32, tag="cs")
            nc.tensor.matmul(loss_ps[:], lv[:], ones128[:],
                             start=True, stop=True)
            losssb = wp.tile([1, 1], F32, tag="losssb")
            nc.scalar.copy(losssb[:], loss_ps[:])
            nc.sync.dma_start(lossp[:], losssb[:])

    return nc


def _split_sync_waits(nc, maxw=1):
    """This walrus build accepts at most one sync-wait command per
    instruction; hoist extra waits onto same-engine nops placed before."""
    import concourse.mybir as mybir

    cnt = 0
    for f in nc.m.functions:
        for bb in f.blocks:
            newlist = []
            for inst in bb.instructions:
                si = inst.sync_info
                if si is not None and si.on_wait is not None \
                        and len(si.on_wait) > maxw:
                    waits = list(si.on_wait)
                    si.on_wait = waits[-maxw:]
                    for w in waits[:-maxw]:
                        cnt += 1
                        nop = mybir.InstNoOp(name=f"I-waitsplit-{cnt}",
                                             ins=[], outs=[])
                        nop.engine = inst.engine
                        nop.sync_info = mybir.SyncInfo(on_wait=[w],
                                                       on_update=[])
                        newlist.append(nop)
                newlist.append(inst)
            if cnt:
                bb.instructions[:] = newlist
    return nc


def _get_compiled():
    if "nc" not in _CACHE:
        _apply_tile_patch()
        _CACHE["nc"] = _split_sync_waits(_build_nc())
    return _CACHE["nc"]


def kernel(soft_input, labels, H, normalizer, _trace=False):
    from concourse import bass_utils

    soft_input = np.asarray(soft_input, dtype=np.float32)
    labels_np = np.asarray(labels)
    H = np.asarray(H)
    normalizer = np.asarray(normalizer, dtype=np.float32)

    hkey = ("tables", H.shape, int(H.sum()), int(H[0].argmax()))
    if hkey not in _CACHE:
        _CACHE[hkey] = _build_tables(H)
    OHT, OHB, CLm = _CACHE[hkey]
    nc = _get_compiled()

    import ml_dtypes
    ident = np.eye(16, dtype=np.float32)
    OHTb = OHT.astype(ml_dtypes.bfloat16)
    CLm2 = np.concatenate(
        [CLm.reshape(128, NSLOT, CL)] * 2, axis=2).reshape(128, NSLOT * 2 * CL)
    CLm2 = np.ascontiguousarray(CLm2, dtype=np.float32)
    gn = normalizer.reshape(1, 1)
    in_maps = []
    for b in range(BATCH):
        in_maps.append({
            "soft16": soft_input[b].reshape(128, 16),
            "lab16": labels_np[b].astype(np.float32).reshape(128, 16),
            "gnorm": gn,
            "OHT": OHTb,
            "OHB": OHB,
            "CLm": CLm,
            "CLm2": CLm2,
            "ident16": ident,
        })

    res = bass_utils.run_bass_kernel_spmd(
        nc, in_maps, core_ids=list(range(BATCH)), trace=_trace)

    stacked = np.zeros((NUM_ITERS + 1, BATCH, COLS), dtype=np.float32)
    stacked[0] = soft_input
    loss = np.float32(0.0)
    for b in range(BATCH):
        stacked[1:, b, :] = res.results[b]["outs"].reshape(NUM_ITERS, COLS)
        loss += np.float32(res.results[b]["lossp"][0, 0])
    out = (stacked, labels_np, np.float32(loss))
    if _trace:
        return out, res
    return out
